# revision 23
# baseline (speedup 1.0000x reference)
"""BiLSTM-CRF negative-log-likelihood kernel for Trainium2 (8 NeuronCores).

Strategy: data-parallel over batch (16 sequences per core), params replicated.
Device computes, per core: the masked emission-score sum (the part of the CRF
numerator that needs emissions) and the CRF partition-function sum (the
denominators).  All label-indexed scalar lookups (start/end/transition scores,
output biases) are tiny and done on host in numpy.
loss = sum_b denom_b - sum_b num_b.

v2 pipeline per core (B_local=16):
  P0: embedding gather (indirect DMA, 128 rows each) + PE transpose -> x^T bf16
      fused with an input pre-GEMM: gx = W_ih x + b for all timesteps, both
      directions, computed chunk-wise (8 timesteps = 512 psum cols, gate-major)
      and scatter-copied to per-t layout [i f o g]x16 in a ring-buffered gxbuf.
  P1: fwd+bwd LSTM scans. All four gates go through ONE Tanh per step
      (i,f,o pre-scaled by 1/2 so sigmoid(z) = (tanh(z/2)+1)/2), cell and
      hidden kept doubled (C=2c, h2=2h) so the whole elementwise tail is
      3 fused scalar_tensor_tensor DVE ops + 1 more Tanh:
        ps   = gx_t + W_hh' h2_{t-1}         (identity-MM + 4 matmuls)
        T'   = tanh(ps)            -> [i' f' o' g']
        X    = (T'[i,f]+1) * [g', C]
        C    = 0.5*X_f + X_i
        tc   = tanh(0.5*C)
        h2   = (o'+1) * tc
      Weights pre-scaled on host (W_hh /2 for h2; i,f,o additionally /2).
  P2: bulk emissions matmul [9, S*16] (w_out pre-halved for h2); fused
      emission-tag reduction against a host-built (one-hot * mask) tensor;
      expem = Exp(em + b_out)
  P3: CRF forward scan in exponential space, tags on partitions: per step one
      9x9 matmul (exp(trans) stationary) + one DVE multiply writing into a
      column history vbuf; periodic renormalisation via PE-transpose dance,
      log corrections recorded in mbuf.
  P4: end-state extraction (indicator multiply + strided reduce), Ln, partial
      sums DMA'd out.
"""

import numpy as np
import ml_dtypes

import concourse.bass as bass
import concourse.bacc as bacc
import concourse.tile as tile
from concourse import mybir
from concourse import bass_utils

F32 = mybir.dt.float32
BF16 = mybir.dt.bfloat16
I32 = mybir.dt.int32

VOCAB, EMB, HID, L = 100000, 128, 256, 9
H = HID // 2  # 128 per direction
B_FULL, S_FULL = 128, 512
N_CORES_FULL = 8
PAD = 0

ALU = mybir.AluOpType
ACTF = mybir.ActivationFunctionType
AXL = mybir.AxisListType


def build_nc(S=S_FULL, BL=16, RN=8, dump=False):
    """Build the per-core Bass program (same program on every core)."""
    assert BL == 16
    NTOK = S * BL                 # tokens per core
    NG = NTOK // 128              # gather groups of 128 tokens == pregemm chunks
    assert NTOK % 128 == 0
    NCH = NTOK // 512             # emission chunks of 512 cols
    assert NTOK % 512 == 0
    NEV = S // RN                 # renorm events
    assert S % RN == 0

    nc = bacc.Bacc("TRN2", target_bir_lowering=False, debug=False)

    # ---- DRAM I/O ----
    d_emb = nc.dram_tensor("emb", [VOCAB, EMB], F32, kind="ExternalInput")
    d_idx = nc.dram_tensor("idx", [128, NG], I32, kind="ExternalInput")
    d_wih = {d: nc.dram_tensor(f"wihT_{d}", [EMB, 4 * H], BF16,
                               kind="ExternalInput") for d in "fb"}
    d_whh = {d: nc.dram_tensor(f"whhT_{d}", [H, 4 * H], BF16,
                               kind="ExternalInput") for d in "fb"}
    d_bias = {d: nc.dram_tensor(f"biasR_{d}", [128, 512], BF16,
                                kind="ExternalInput") for d in "fb"}
    d_wout = {d: nc.dram_tensor(f"woutT_{d}", [H, L], BF16,
                                kind="ExternalInput") for d in "fb"}
    d_idf = nc.dram_tensor("ident_f32", [128, 128], F32, kind="ExternalInput")
    d_idb = nc.dram_tensor("ident_bf16", [128, 128], BF16,
                           kind="ExternalInput")
    M = S // 2                    # alpha/beta split point
    NB = S - M                    # beta steps
    NEVA = (M + RN - 1) // RN     # alpha renorm events
    NEVB = (NB + RN - 1) // RN    # beta renorm events
    d_teC = nc.dram_tensor("te9C", [L, L], F32, kind="ExternalInput")
    d_teCT = nc.dram_tensor("te9CT", [L, L], F32, kind="ExternalInput")
    d_estart = nc.dram_tensor("expstart", [L, 1], F32, kind="ExternalInput")
    d_eend = nc.dram_tensor("expend", [L, 1], F32, kind="ExternalInput")
    d_bout = nc.dram_tensor("bout9", [L, 1], F32, kind="ExternalInput")
    d_ones9 = nc.dram_tensor("ones9", [L, 1], F32, kind="ExternalInput")
    d_ones16 = nc.dram_tensor("ones16", [16, 1], F32, kind="ExternalInput")
    d_ohm = nc.dram_tensor("ohm", [L, NTOK], F32, kind="ExternalInput")
    d_injbar = nc.dram_tensor("injbar", [L, 16 * NB], F32,
                              kind="ExternalInput")
    d_einj = nc.dram_tensor("einj", [L, 16 * NB], F32, kind="ExternalInput")
    d_i255b = nc.dram_tensor("i255b", [L, 16], F32, kind="ExternalInput")
    d_ei255 = nc.dram_tensor("ei255", [L, 16], F32, kind="ExternalInput")
    d_indEB = nc.dram_tensor("indEB", [16, NEVB], F32, kind="ExternalInput")
    d_out = nc.dram_tensor("out2", [1, 2], F32, kind="ExternalOutput")
    if dump:
        d_xd = nc.dram_tensor("xd", [128, S * 16], BF16,
                              kind="ExternalOutput")
        d_hd = {d: nc.dram_tensor(f"hd_{d}", [H, S * 16], BF16,
                                  kind="ExternalOutput") for d in "fb"}
        d_exd = nc.dram_tensor("exd", [L, S * 16], F32,
                               kind="ExternalOutput")

    with tile.TileContext(nc) as tc:
        persist = tc.alloc_tile_pool(name="persist", bufs=1)

        # ---- persistent small tensors ----
        idx_t = persist.tile([128, NG], I32, name="idx_t")
        nc.sync.dma_start(idx_t[:], d_idx[:])
        wih, whh, biasR, wout = {}, {}, {}, {}
        for d in "fb":
            wih[d] = persist.tile([EMB, 4 * H], BF16, name=f"wih_{d}")
            nc.sync.dma_start(wih[d][:], d_wih[d][:])
            whh[d] = persist.tile([H, 4 * H], BF16, name=f"whh_{d}")
            nc.sync.dma_start(whh[d][:], d_whh[d][:])
            biasR[d] = persist.tile([128, 512], BF16, name=f"biasR_{d}")
            nc.sync.dma_start(biasR[d][:], d_bias[d][:])
            wout[d] = persist.tile([H, L], BF16, name=f"wout_{d}")
            nc.sync.dma_start(wout[d][:], d_wout[d][:])
        idf = persist.tile([128, 128], F32, name="idf")
        nc.sync.dma_start(idf[:], d_idf[:])
        idb = persist.tile([128, 128], BF16, name="idb")
        nc.sync.dma_start(idb[:], d_idb[:])
        teC = persist.tile([L, L], F32, name="teC_t")
        nc.sync.dma_start(teC[:], d_teC[:])
        teCT = persist.tile([L, L], F32, name="teCT_t")
        nc.sync.dma_start(teCT[:], d_teCT[:])
        estart = persist.tile([L, 1], F32, name="estart_t")
        nc.sync.dma_start(estart[:], d_estart[:])
        eend = persist.tile([L, 1], F32, name="eend_t")
        nc.sync.dma_start(eend[:], d_eend[:])
        bout = persist.tile([L, 1], F32, name="bout_t")
        nc.sync.dma_start(bout[:], d_bout[:])
        ones9 = persist.tile([L, 1], F32, name="ones9_t")
        nc.sync.dma_start(ones9[:], d_ones9[:])
        ones16 = persist.tile([16, 1], F32, name="ones16_t")
        nc.sync.dma_start(ones16[:], d_ones16[:])
        mbufA = persist.tile([16, NEVA], F32, name="mbufA")
        mbufB = persist.tile([16, NEVB], F32, name="mbufB")
        indEB = persist.tile([16, NEVB], F32, name="indEB_t")
        nc.sync.dma_start(indEB[:], d_indEB[:])
        emacc = persist.tile([L, NCH], F32, name="emacc")
        out_sb = persist.tile([1, 2], F32, name="out_sb")

        # ---- scan-side persistent buffers (allocated first: stable addrs) --
        pool_h = tc.alloc_tile_pool(name="hpool", bufs=1, side="right")
        hbuf = {d: pool_h.tile([H, NTOK], BF16, name=f"hbuf_{d}")
                for d in "fb"}
        pool_U = tc.alloc_tile_pool(name="Upool", bufs=1, side="right")
        U = {d: pool_U.tile([128, 96], F32, name=f"U_{d}") for d in "fb"}
        pool_X = tc.alloc_tile_pool(name="Xpool", bufs=3, side="right")

        # scan PSUM first so its banks never alias gather/pregemm banks
        pool_sps = tc.alloc_tile_pool(name="scanps", bufs=2, space="PSUM")

        # ======== Phase 0: gather + transpose + input pre-GEMM =============
        # full xT (both pregemm streams need chunks long after gather);
        # gx ring-buffered (8 chunks per dir, consumed in production order)
        pool_xw = tc.alloc_tile_pool(name="xtpool", bufs=1)
        xT = pool_xw.tile([128, NTOK], BF16, name="xT")
        pool_gx = tc.alloc_tile_pool(name="gxwin", bufs=8)
        pool_g = tc.alloc_tile_pool(name="gpool", bufs=4)
        pool_gp = tc.alloc_tile_pool(name="gppool", bufs=2, space="PSUM")
        pool_pgps = tc.alloc_tile_pool(name="pregemmps", bufs=2, space="PSUM")

        # interleave gather order so both scan directions can start early
        g_order = []
        lo, hi = 0, NG - 1
        while lo <= hi:
            g_order.append(lo)
            if hi != lo:
                g_order.append(hi)
            lo += 1
            hi -= 1
        gxt = {d: {} for d in "fb"}   # chunk g -> gx tile [128, 512] bf16

        def emit_gather(g):
            with nc.named_scope("p0_gather"):
                stage = pool_g.tile([128, EMB], F32, name="stage", tag="stage")
                nc.gpsimd.indirect_dma_start(
                    out=stage[:],
                    out_offset=None,
                    in_=d_emb[:],
                    in_offset=bass.IndirectOffsetOnAxis(ap=idx_t[:, g:g + 1],
                                                        axis=0),
                )
                tp = pool_gp.tile([128, 128], F32, name="tp", tag="tp")
                nc.tensor.transpose(out=tp[:], in_=stage[:], identity=idf[:])
                nc.vector.tensor_copy(out=xT[:, 128 * g:128 * (g + 1)],
                                      in_=tp[:])

        def emit_pregemm(d, g):
            with nc.named_scope("p0_pregemm"):
                pg = pool_pgps.tile([128, 512], F32, name="pg", tag="pg")
                # bias init (gate-major layout), then 4 gate blocks
                nc.tensor.matmul(out=pg[:], lhsT=idb[:], rhs=biasR[d][:],
                                 start=True, stop=False)
                for k in range(4):
                    nc.tensor.matmul(
                        out=pg[:, 128 * k:128 * (k + 1)],
                        lhsT=wih[d][:, 128 * k:128 * (k + 1)],
                        rhs=xT[:, 128 * g:128 * (g + 1)],
                        start=False, stop=(k == 3))
                gx = pool_gx.tile([128, 512], BF16, name=f"gx{d}",
                                  tag=f"gx{d}")
                gxt[d][g] = gx
                # keep gate-major layout; scan reads a strided rhs AP
                nc.vector.tensor_copy(out=gx[:], in_=pg[:])

        # warmup: gather first 12 x-chunks, pregemm first 4 per direction.
        # GWARM=12 keeps both pregemm streams >= 12 timesteps behind their
        # gathers (b-stream chunk 59-t/8 needs g_order position 2(63-c)+1).
        GWARM = min(12, NG)
        PWARM = min(4, NG // 2)
        for j in range(GWARM):
            emit_gather(g_order[j])
        for j in range(PWARM):
            emit_pregemm("f", j)
            emit_pregemm("b", NG - 1 - j)

        # ================= Phase 1: dual LSTM scan ==========================
        for d in "fb":
            nc.vector.memset(U[d][:, 64:80], 0.0)  # C = 2c = 0

        for t in range(S):
            if t % 4 == 0 and GWARM + t // 4 < NG:
                emit_gather(g_order[GWARM + t // 4])
            if t % 8 == 0:
                if PWARM + t // 8 < NG:
                    emit_pregemm("f", PWARM + t // 8)
                if NG - 1 - PWARM - t // 8 >= 0:
                    emit_pregemm("b", NG - 1 - PWARM - t // 8)
            with nc.named_scope("p1_scan"):
                for d in "fb":
                    tx = t if d == "f" else S - 1 - t  # absolute time index
                    txp = tx - 1 if d == "f" else tx + 1
                    g = tx // 8
                    gxg = gxt[d][g]
                    o8 = 16 * (tx % 8)
                    ps = pool_sps.tile([128, 64], F32, name=f"ps_{d}",
                                       tag=f"ps{d}")
                    for k in range(4):
                        nc.tensor.matmul(
                            out=ps[:, 16 * k:16 * (k + 1)],
                            lhsT=idb[:],
                            rhs=gxg[:, 128 * k + o8:128 * k + o8 + 16],
                            start=True, stop=(t == 0 and k == 3))
                    if t > 0:
                        hp = hbuf[d][:, 16 * txp:16 * txp + 16]
                        for k in range(4):
                            nc.tensor.matmul(
                                out=ps[:, 16 * k:16 * (k + 1)],
                                lhsT=whh[d][:, 128 * k:128 * (k + 1)],
                                rhs=hp,
                                start=False, stop=(k == 3))
                    # T' = tanh(ps) = [i' f' o' g']
                    nc.scalar.activation(U[d][:, 0:64], ps[:], ACTF.Tanh)
                    X = pool_X.tile([128, 32], F32, name="X", tag="X")
                    # X = (T'[i,f] + 1) * [g', C]
                    nc.vector.scalar_tensor_tensor(
                        out=X[:], in0=U[d][:, 0:32], scalar=1.0,
                        in1=U[d][:, 48:80], op0=ALU.add, op1=ALU.mult)
                    # C = 0.5*X_f + X_i
                    nc.vector.scalar_tensor_tensor(
                        out=U[d][:, 64:80], in0=X[:, 16:32], scalar=0.5,
                        in1=X[:, 0:16], op0=ALU.mult, op1=ALU.add)
                    # tc = tanh(0.5*C)
                    nc.scalar.activation(U[d][:, 80:96], U[d][:, 64:80],
                                         ACTF.Tanh, scale=0.5)
                    # h2 = (o' + 1) * tc
                    nc.vector.scalar_tensor_tensor(
                        out=hbuf[d][:, 16 * tx:16 * tx + 16],
                        in0=U[d][:, 32:48], scalar=1.0,
                        in1=U[d][:, 80:96], op0=ALU.add, op1=ALU.mult)

        if dump:
            nc.sync.dma_start(d_xd[:], xT[:])
        pool_pgps.release()
        pool_gp.release()
        pool_g.release()
        pool_gx.release()
        pool_xw.release()
        pool_sps.release()
        pool_X.release()
        pool_U.release()

        # ============= Phase 2: emissions + em_tag + exp ================
        pool_em = tc.alloc_tile_pool(name="empool", bufs=1)
        expem = pool_em.tile([L, NTOK], F32, name="expem")
        pool_ohm = tc.alloc_tile_pool(name="ohmpool", bufs=1)
        ohm_t = pool_ohm.tile([L, NTOK], F32, name="ohm_t")
        nc.sync.dma_start(ohm_t[:], d_ohm[:])
        pool_er = tc.alloc_tile_pool(name="emrot", bufs=2)
        pool_eps = tc.alloc_tile_pool(name="emps", bufs=2, space="PSUM")
        for c in range(NCH):
            with nc.named_scope("p2_emis"):
                sl = slice(512 * c, 512 * (c + 1))
                pe = pool_eps.tile([L, 512], F32, name="pe", tag="pe")
                nc.tensor.matmul(out=pe[:], lhsT=wout["f"][:],
                                 rhs=hbuf["f"][:, sl], start=True, stop=False)
                nc.tensor.matmul(out=pe[:], lhsT=wout["b"][:],
                                 rhs=hbuf["b"][:, sl], start=False, stop=True)
                scr = pool_er.tile([L, 512], F32, name="scr", tag="scr")
                nc.vector.tensor_tensor(out=scr[:], in0=pe[:],
                                        in1=ohm_t[:, sl], op=ALU.mult)
                nc.vector.tensor_reduce(out=emacc[:, c:c + 1], in_=scr[:],
                                        axis=AXL.X, op=ALU.add)
                nc.scalar.activation(expem[:, sl], pe[:], ACTF.Exp,
                                     bias=bout[:])

        emaccs = pool_er.tile([L, 1], F32, name="emaccs", tag="emaccs")
        nc.vector.tensor_reduce(out=emaccs[:], in_=emacc[:], axis=AXL.X,
                                op=ALU.add)
        pss = pool_eps.tile([1, 1], F32, name="pss", tag="pss")
        nc.tensor.matmul(out=pss[:], lhsT=ones9[:], rhs=emaccs[:],
                         start=True, stop=True)
        nc.vector.tensor_copy(out=out_sb[:, 0:1], in_=pss[:])

        pool_eps.release()
        pool_er.release()
        pool_ohm.release()
        if dump:
            for d in "fb":
                nc.sync.dma_start(d_hd[d][:], hbuf[d][:])
            nc.sync.dma_start(d_exd[:], expem[:])
        pool_h.release()

        # ===== Phase 3: CRF alpha/beta split scan (exp space, C-folded) =====
        MB = 16 * NB
        pool_ab = tc.alloc_tile_pool(name="abpool", bufs=1)
        Abuf = pool_ab.tile([L, MB], F32, name="Abuf")
        TBbuf = pool_ab.tile([L, MB], F32, name="TBbuf")
        Btmp = pool_ab.tile([L, MB], F32, name="Btmp")
        injbar_t = pool_ab.tile([L, MB], F32, name="injbar_t")
        einj_t = pool_ab.tile([L, MB], F32, name="einj_t")
        i255b_t = pool_ab.tile([L, 16], F32, name="i255b_t")
        ei255_t = pool_ab.tile([L, 16], F32, name="ei255_t")
        nc.sync.dma_start(injbar_t[:], d_injbar[:])
        nc.sync.dma_start(einj_t[:], d_einj[:])
        nc.sync.dma_start(i255b_t[:], d_i255b[:])
        nc.sync.dma_start(ei255_t[:], d_ei255[:])
        pool_cps = tc.alloc_tile_pool(name="crfps", bufs=2, space="PSUM")
        pool_cps1 = tc.alloc_tile_pool(name="crfps1", bufs=1, space="PSUM")
        pool_cr = tc.alloc_tile_pool(name="crfrot", bufs=3, side="right")

        with nc.named_scope("p3_pre"):
            # A = e*(1-inj);  B = e*E*inj;  TB = Te' @ B
            nc.vector.tensor_tensor(out=Abuf[:], in0=expem[:, 16 * M:],
                                    in1=injbar_t[:], op=ALU.mult)
            nc.vector.tensor_tensor(out=Btmp[:], in0=expem[:, 16 * M:],
                                    in1=einj_t[:], op=ALU.mult)
            for c in range(MB // 512):
                tb_ps = pool_cps1.tile([L, 512], F32, name="tb_ps",
                                       tag="tbps")
                nc.tensor.matmul(out=tb_ps[:], lhsT=teCT[:],
                                 rhs=Btmp[:, 512 * c:512 * (c + 1)],
                                 start=True, stop=True)
                nc.vector.tensor_copy(out=TBbuf[:, 512 * c:512 * (c + 1)],
                                      in_=tb_ps[:])

        def renorm(vtile, mtile, e, tagp):
            vt_ps = pool_cps1.tile([16, L], F32, name="vt_ps", tag="vtps")
            nc.tensor.transpose(out=vt_ps[:], in_=vtile[:],
                                identity=idf[0:L, 0:L])
            nc.vector.tensor_reduce(out=mtile[:, e:e + 1], in_=vt_ps[:],
                                    axis=AXL.X, op=ALU.max)
            rt = pool_cr.tile([16, 1], F32, name="rt", tag="rt")
            nc.vector.reciprocal(out=rt[:], in_=mtile[:, e:e + 1])
            vts = pool_cr.tile([16, L], F32, name="vts", tag="vts")
            nc.vector.tensor_scalar(out=vts[:], in0=vt_ps[:],
                                    scalar1=rt[:], scalar2=None, op0=ALU.mult)
            v2_ps = pool_cps1.tile([L, 16], F32, name="v2_ps", tag="v2ps")
            nc.tensor.transpose(out=v2_ps[:], in_=vts[:],
                                identity=idf[0:16, 0:16])
            out = pool_cr.tile([L, 16], F32, name="rn_out", tag=tagp)
            nc.vector.tensor_copy(out=out[:], in_=v2_ps[:])
            return out

        # ---- alpha chain: t = 1 .. M-1 (no masking: all lens >= M) ----
        va = pool_cr.tile([L, 16], F32, name="va", tag="va")
        nc.vector.tensor_scalar(out=va[:], in0=expem[:, 0:16],
                                scalar1=estart[:], scalar2=None, op0=ALU.mult)
        # ---- beta chain init: u = exp(end) ----
        ub0 = pool_cr.tile([L, 16], F32, name="ub0", tag="ub")
        nc.vector.memset(ub0[:], 1.0)
        ub = pool_cr.tile([L, 16], F32, name="ub", tag="ub")
        nc.vector.tensor_scalar(out=ub[:], in0=ub0[:], scalar1=eend[:],
                                scalar2=None, op0=ALU.mult)

        for step in range(1, max(M, NB + 1)):
            ta = step                 # alpha time index
            s = step - 1              # beta step index; t = S-1-s
            if ta < M:
                with nc.named_scope("p3_alpha"):
                    a_ps = pool_cps.tile([L, 16], F32, name="a_ps", tag="aps")
                    nc.tensor.matmul(out=a_ps[:], lhsT=teC[:], rhs=va[:],
                                     start=True, stop=True)
                    va2 = pool_cr.tile([L, 16], F32, name="va2", tag="va")
                    nc.vector.tensor_tensor(
                        out=va2[:], in0=a_ps[:],
                        in1=expem[:, 16 * ta:16 * ta + 16], op=ALU.mult)
                    va = va2
                    if ta % RN == RN - 1:
                        va = renorm(va, mbufA, ta // RN, "va")
            if s < NB:
                tb = S - 1 - s
                off = 16 * (tb - M)
                with nc.named_scope("p3_beta"):
                    w = pool_cr.tile([L, 16], F32, name="w", tag="wb")
                    nc.vector.tensor_tensor(out=w[:], in0=ub[:],
                                            in1=Abuf[:, off:off + 16],
                                            op=ALU.mult)
                    b_ps = pool_cps.tile([L, 16], F32, name="b_ps", tag="bps")
                    nc.tensor.matmul(out=b_ps[:], lhsT=teCT[:], rhs=w[:],
                                     start=True, stop=True)
                    ub2 = pool_cr.tile([L, 16], F32, name="ub2", tag="ub")
                    nc.vector.tensor_tensor(out=ub2[:], in0=b_ps[:],
                                            in1=TBbuf[:, off:off + 16],
                                            op=ALU.add)
                    ub = ub2
                    if s % RN == RN - 1:
                        ub = renorm(ub, mbufB, s // RN, "ub")

        pool_cps1.release()
        pool_cps.release()

        # ============= Phase 4: combine + finals ==============================
        _p4 = nc.named_scope("p4_final")
        _p4.__enter__()
        pool_f4 = tc.alloc_tile_pool(name="f4", bufs=1)
        pool_fps = tc.alloc_tile_pool(name="f4ps", bufs=2, space="PSUM")
        # boundary fix for len-1 == M-1 columns, then P = va (.) u
        uf1 = pool_f4.tile([L, 16], F32, name="uf1")
        nc.vector.tensor_tensor(out=uf1[:], in0=ub[:], in1=i255b_t[:],
                                op=ALU.mult)
        ufix = pool_f4.tile([L, 16], F32, name="ufix")
        nc.vector.tensor_tensor(out=ufix[:], in0=uf1[:], in1=ei255_t[:],
                                op=ALU.add)
        P = pool_f4.tile([L, 16], F32, name="P")
        nc.vector.tensor_tensor(out=P[:], in0=va[:], in1=ufix[:],
                                op=ALU.mult)
        w_ps = pool_fps.tile([1, 16], F32, name="w_ps", tag="wps")
        nc.tensor.matmul(out=w_ps[:], lhsT=ones9[:], rhs=P[:],
                         start=True, stop=True)
        lw = pool_f4.tile([1, 16], F32, name="lw")
        nc.scalar.activation(lw[:], w_ps[:], ACTF.Ln)
        lwT_ps = pool_fps.tile([16, 1], F32, name="lwT_ps", tag="lwT")
        nc.tensor.transpose(out=lwT_ps[:], in_=lw[:],
                            identity=idf[0:1, 0:1])
        lnA = pool_f4.tile([16, NEVA], F32, name="lnA")
        nc.scalar.activation(lnA[:], mbufA[:], ACTF.Ln)
        redA = pool_f4.tile([16, 1], F32, name="redA")
        nc.vector.tensor_reduce(out=redA[:], in_=lnA[:], axis=AXL.X,
                                op=ALU.add)
        lnB = pool_f4.tile([16, NEVB], F32, name="lnB")
        nc.scalar.activation(lnB[:], mbufB[:], ACTF.Ln)
        lnBm = pool_f4.tile([16, NEVB], F32, name="lnBm")
        nc.vector.tensor_tensor(out=lnBm[:], in0=lnB[:], in1=indEB[:],
                                op=ALU.mult)
        redB = pool_f4.tile([16, 1], F32, name="redB")
        nc.vector.tensor_reduce(out=redB[:], in_=lnBm[:], axis=AXL.X,
                                op=ALU.add)
        dst = pool_f4.tile([16, 1], F32, name="dst")
        nc.vector.tensor_tensor(out=dst[:], in0=lwT_ps[:], in1=redA[:],
                                op=ALU.add)
        dst2 = pool_f4.tile([16, 1], F32, name="dst2")
        nc.vector.tensor_tensor(out=dst2[:], in0=dst[:], in1=redB[:],
                                op=ALU.add)
        dtot_ps = pool_fps.tile([1, 1], F32, name="dtot_ps", tag="dtot")
        nc.tensor.matmul(out=dtot_ps[:], lhsT=ones16[:], rhs=dst2[:],
                         start=True, stop=True)
        nc.vector.tensor_copy(out=out_sb[:, 1:2], in_=dtot_ps[:])
        pool_fps.release()
        pool_f4.release()
        _p4.__exit__(None, None, None)
        pool_cr.release()
        pool_ab.release()
        pool_em.release()

        nc.sync.dma_start(d_out[:], out_sb[:])
        persist.release()

    nc.compile()
    return nc


# ---------------------------------------------------------------------------
# Host side
# ---------------------------------------------------------------------------

def _prep_core_inputs(core, seqs, labels, start_t, end_t, trans, b_out,
                      S, BL, RN, lnC, shared):
    NTOK = S * BL
    NG = NTOK // 128
    M = S // 2
    NB = S - M
    NEVB = NB // RN
    b0 = core * BL
    sq = seqs[b0:b0 + BL]          # [BL, S]
    lb = labels[b0:b0 + BL]
    lens = (sq != PAD).sum(axis=1).astype(np.int64)
    maskf = (sq != PAD).astype(np.float32)

    # token gather indices in (t, b) order
    toks = sq.T.reshape(-1).astype(np.int32)       # [S*BL], t-major
    idx = np.ascontiguousarray(toks.reshape(NG, 128).T)

    ohm = np.zeros((L, NTOK), np.float32)
    cols = np.arange(NTOK)
    t_of = cols // BL
    b_of = cols % BL
    ohm[lb[b_of, t_of], cols] = maskf[b_of, t_of]

    # beta-chain injection tensors: inj_t[b] = (len_b-1 == t), t in [M, S-1]
    E = np.exp(end_t.astype(np.float32))           # [L]
    ts = np.arange(M, S)
    inj = (lens[None, :] - 1 == ts[:, None]).astype(np.float32)  # [NB, BL]
    injbar = np.ascontiguousarray(
        np.broadcast_to((1.0 - inj).reshape(1, -1), (L, 16 * NB)).astype(
            np.float32))
    einj = np.ascontiguousarray(
        (E[:, None, None] * inj[None]).reshape(L, 16 * NB).astype(np.float32))
    i255 = (lens - 1 == M - 1).astype(np.float32)  # [BL]
    i255b = np.ascontiguousarray(
        np.broadcast_to((1.0 - i255)[None, :], (L, 16)).astype(np.float32))
    ei255 = np.ascontiguousarray(
        (E[:, None] * i255[None, :]).astype(np.float32))
    # beta renorm event at s=e*RN+RN-1 -> t_e = S-1-s; survives iff
    # t_e <= len_b-1
    s_e = np.arange(NEVB) * RN + RN - 1
    t_e = S - 1 - s_e
    indEB = np.ascontiguousarray(
        (t_e[None, :] <= (lens - 1)[:, None]).astype(np.float32))

    inmap = dict(shared)
    inmap["idx"] = idx
    inmap["ohm"] = ohm
    inmap["injbar"] = injbar
    inmap["einj"] = einj
    inmap["i255b"] = i255b
    inmap["ei255"] = ei255
    inmap["indEB"] = indEB

    ar = np.arange(BL)
    hostnum = (start_t[lb[:, 0]]
               + (trans[lb[:, :-1], lb[:, 1:]] * maskf[:, 1:]).sum(axis=1)
               + end_t[lb[ar, lens - 1]]
               + (maskf * b_out[lb]).sum(axis=1))
    # C-fold correction: device denom includes (len-1)*lnC extra
    hostnum_total = float(hostnum.sum()) + lnC * float((lens - 1).sum())
    return inmap, hostnum_total


def _shared_inputs(emb, w_ih, w_hh, b_ih, b_hh, w_out, b_out, start_t,
                   end_t, trans):
    # pytorch gate rows [i, f, g, o] -> device gate blocks [i, f, o, g]
    perm = [0, 1, 3, 2]
    # tanh-only gates: i,f,o pre-scaled by 1/2 (sigmoid via tanh); h stored
    # as h2=2h so all W_hh contributions halved again.
    sc_ih = [0.5, 0.5, 0.5, 1.0]
    sc_hh = [0.25, 0.25, 0.25, 0.5]

    def wprep(w, scales):  # [4H, K] -> [K, 4H] col blocks in perm order
        blocks = [w[128 * p:128 * (p + 1)].T * s
                  for p, s in zip(perm, scales)]
        return np.ascontiguousarray(
            np.concatenate(blocks, axis=1)).astype(ml_dtypes.bfloat16)

    def bprep(bi, bh):
        bsum = (bi + bh).astype(np.float32)
        blocks = [np.repeat(bsum[128 * p:128 * (p + 1)][:, None] * s,
                            128, axis=1)
                  for p, s in zip(perm, sc_ih)]
        return np.ascontiguousarray(
            np.concatenate(blocks, axis=1)).astype(ml_dtypes.bfloat16)

    Te = np.exp(trans.astype(np.float64))
    C = float(1.0 / (L * Te.mean() * np.exp(b_out.astype(np.float64)).mean()))
    lnC = float(np.log(C))
    te9C = (C * Te).astype(np.float32)

    shared = {
        "emb": np.ascontiguousarray(emb, dtype=np.float32),
        "ident_f32": np.eye(128, dtype=np.float32),
        "ident_bf16": np.eye(128).astype(ml_dtypes.bfloat16),
        "te9C": np.ascontiguousarray(te9C),
        "te9CT": np.ascontiguousarray(te9C.T),
        "expstart": np.exp(start_t.astype(np.float32))[:, None].copy(),
        "expend": np.exp(end_t.astype(np.float32))[:, None].copy(),
        "bout9": b_out.astype(np.float32)[:, None].copy(),
        "ones9": np.ones((L, 1), np.float32),
        "ones16": np.ones((16, 1), np.float32),
    }
    for d in "fb":
        shared[f"wihT_{d}"] = wprep(w_ih[d], sc_ih)
        shared[f"whhT_{d}"] = wprep(w_hh[d], sc_hh)
        shared[f"biasR_{d}"] = bprep(b_ih[d], b_hh[d])
    shared["woutT_f"] = np.ascontiguousarray(
        0.5 * w_out[:, :H].T).astype(ml_dtypes.bfloat16)
    shared["woutT_b"] = np.ascontiguousarray(
        0.5 * w_out[:, H:].T).astype(ml_dtypes.bfloat16)
    return shared, lnC


_CACHE = {}


def run(inputs, S=S_FULL, BL=16, RN=None, n_cores=N_CORES_FULL, dump=False,
        **spmd_kwargs):
    seqs = np.asarray(inputs["sequences"])
    labels = np.asarray(inputs["labels"])
    emb = np.asarray(inputs["emb"], np.float32)
    w_ih = {"f": np.asarray(inputs["w_ih_f"], np.float32),
            "b": np.asarray(inputs["w_ih_b"], np.float32)}
    w_hh = {"f": np.asarray(inputs["w_hh_f"], np.float32),
            "b": np.asarray(inputs["w_hh_b"], np.float32)}
    b_ih = {"f": np.asarray(inputs["b_ih_f"], np.float32),
            "b": np.asarray(inputs["b_ih_b"], np.float32)}
    b_hh = {"f": np.asarray(inputs["b_hh_f"], np.float32),
            "b": np.asarray(inputs["b_hh_b"], np.float32)}
    w_out = np.asarray(inputs["w_out"], np.float32)
    b_out = np.asarray(inputs["b_out"], np.float32)
    start_t = np.asarray(inputs["start_t"], np.float32)
    end_t = np.asarray(inputs["end_t"], np.float32)
    trans = np.asarray(inputs["trans"], np.float32)

    if RN is None:
        RN = 64 if S >= 512 else 16

    key = (S, BL, RN, dump)
    if key not in _CACHE:
        _CACHE[key] = build_nc(S=S, BL=BL, RN=RN, dump=dump)
    nc = _CACHE[key]

    shared, lnC = _shared_inputs(emb, w_ih, w_hh, b_ih, b_hh, w_out, b_out,
                                 start_t, end_t, trans)
    in_maps = []
    hostnum_total = 0.0
    for c in range(n_cores):
        im, hn = _prep_core_inputs(c, seqs, labels, start_t, end_t, trans,
                                   b_out, S, BL, RN, lnC, shared)
        in_maps.append(im)
        hostnum_total += hn

    res = bass_utils.run_bass_kernel_spmd(nc, in_maps,
                                          core_ids=list(range(n_cores)),
                                          **spmd_kwargs)
    emtag_total = 0.0
    denom_total = 0.0
    for r in res.results:
        emtag_total += float(r["out2"][0, 0])
        denom_total += float(r["out2"][0, 1])
    loss = denom_total - (hostnum_total + emtag_total)
    return np.array(loss, dtype=np.float32), res


def kernel(**inputs):
    loss, _ = run(inputs)
    return loss


# revision 24
# speedup vs baseline: 1.1999x; 1.1999x over previous
"""BiLSTM-CRF negative-log-likelihood kernel for Trainium2 (8 NeuronCores).

Strategy: data-parallel over batch (16 sequences per core), params replicated.
Device computes, per core: the masked emission-score sum (the part of the CRF
numerator that needs emissions) and the CRF partition-function sum (the
denominators).  All label-indexed scalar lookups (start/end/transition scores,
output biases) are tiny and done on host in numpy.
loss = sum_b denom_b - sum_b num_b.

v2 pipeline per core (B_local=16):
  P0: embedding gather (indirect DMA, 128 rows each) + PE transpose -> x^T bf16
      fused with an input pre-GEMM: gx = W_ih x + b for all timesteps, both
      directions, computed chunk-wise (8 timesteps = 512 psum cols, gate-major)
      and scatter-copied to per-t layout [i f o g]x16 in a ring-buffered gxbuf.
  P1: fwd+bwd LSTM scans. All four gates go through ONE Tanh per step
      (i,f,o pre-scaled by 1/2 so sigmoid(z) = (tanh(z/2)+1)/2), cell and
      hidden kept doubled (C=2c, h2=2h) so the whole elementwise tail is
      3 fused scalar_tensor_tensor DVE ops + 1 more Tanh:
        ps   = gx_t + W_hh' h2_{t-1}         (identity-MM + 4 matmuls)
        T'   = tanh(ps)            -> [i' f' o' g']
        X    = (T'[i,f]+1) * [g', C]
        C    = 0.5*X_f + X_i
        tc   = tanh(0.5*C)
        h2   = (o'+1) * tc
      Weights pre-scaled on host (W_hh /2 for h2; i,f,o additionally /2).
  P2: bulk emissions matmul [9, S*16] (w_out pre-halved for h2); fused
      emission-tag reduction against a host-built (one-hot * mask) tensor;
      expem = Exp(em + b_out)
  P3: CRF forward scan in exponential space, tags on partitions: per step one
      9x9 matmul (exp(trans) stationary) + one DVE multiply writing into a
      column history vbuf; periodic renormalisation via PE-transpose dance,
      log corrections recorded in mbuf.
  P4: end-state extraction (indicator multiply + strided reduce), Ln, partial
      sums DMA'd out.
"""

import numpy as np
import ml_dtypes

import concourse.bass as bass
import concourse.bacc as bacc
import concourse.tile as tile
from concourse import mybir
from concourse import bass_utils

F32 = mybir.dt.float32
BF16 = mybir.dt.bfloat16
I32 = mybir.dt.int32

VOCAB, EMB, HID, L = 100000, 128, 256, 9
H = HID // 2  # 128 per direction
B_FULL, S_FULL = 128, 512
N_CORES_FULL = 8
PAD = 0

ALU = mybir.AluOpType
ACTF = mybir.ActivationFunctionType
AXL = mybir.AxisListType


def build_nc(S=S_FULL, BL=16, RN=8, dump=False):
    """Build the per-core Bass program (same program on every core)."""
    assert BL == 16
    NTOK = S * BL                 # tokens per core
    NG = NTOK // 128              # gather groups of 128 tokens == pregemm chunks
    assert NTOK % 128 == 0
    NCH = NTOK // 512             # emission chunks of 512 cols
    assert NTOK % 512 == 0
    NEV = S // RN                 # renorm events
    assert S % RN == 0

    nc = bacc.Bacc("TRN2", target_bir_lowering=False, debug=False)

    # ---- DRAM I/O ----
    d_emb = nc.dram_tensor("emb", [VOCAB, EMB], F32, kind="ExternalInput")
    d_idx = nc.dram_tensor("idx", [128, NG], I32, kind="ExternalInput")
    d_wih = {d: nc.dram_tensor(f"wihT_{d}", [EMB, 4 * H], BF16,
                               kind="ExternalInput") for d in "fb"}
    d_whh = {d: nc.dram_tensor(f"whhT_{d}", [H, 4 * H], BF16,
                               kind="ExternalInput") for d in "fb"}
    d_bias = {d: nc.dram_tensor(f"biasR_{d}", [128, 512], BF16,
                                kind="ExternalInput") for d in "fb"}
    d_wout = {d: nc.dram_tensor(f"woutT_{d}", [H, L], BF16,
                                kind="ExternalInput") for d in "fb"}
    d_idf = nc.dram_tensor("ident_f32", [128, 128], F32, kind="ExternalInput")
    d_idb = nc.dram_tensor("ident_bf16", [128, 128], BF16,
                           kind="ExternalInput")
    M = S // 2                    # alpha/beta split point
    NB = S - M                    # beta steps
    NEVA = (M + RN - 1) // RN     # alpha renorm events
    NEVB = (NB + RN - 1) // RN    # beta renorm events
    d_teC = nc.dram_tensor("te9C", [L, L], F32, kind="ExternalInput")
    d_teCT = nc.dram_tensor("te9CT", [L, L], F32, kind="ExternalInput")
    d_estart = nc.dram_tensor("expstart", [L, 1], F32, kind="ExternalInput")
    d_eend = nc.dram_tensor("expend", [L, 1], F32, kind="ExternalInput")
    d_bout = nc.dram_tensor("bout9", [L, 1], F32, kind="ExternalInput")
    d_ones9 = nc.dram_tensor("ones9", [L, 1], F32, kind="ExternalInput")
    d_ones16 = nc.dram_tensor("ones16", [16, 1], F32, kind="ExternalInput")
    d_ohm = nc.dram_tensor("ohm", [L, NTOK], F32, kind="ExternalInput")
    d_injbar = nc.dram_tensor("injbar", [L, 16 * NB], F32,
                              kind="ExternalInput")
    d_einj = nc.dram_tensor("einj", [L, 16 * NB], F32, kind="ExternalInput")
    d_i255b = nc.dram_tensor("i255b", [L, 16], F32, kind="ExternalInput")
    d_ei255 = nc.dram_tensor("ei255", [L, 16], F32, kind="ExternalInput")
    d_indEB = nc.dram_tensor("indEB", [16, NEVB], F32, kind="ExternalInput")
    d_out = nc.dram_tensor("out2", [1, 2], F32, kind="ExternalOutput")
    if dump:
        d_xd = nc.dram_tensor("xd", [128, S * 16], BF16,
                              kind="ExternalOutput")
        d_hd = {d: nc.dram_tensor(f"hd_{d}", [H, S * 16], BF16,
                                  kind="ExternalOutput") for d in "fb"}
        d_exd = nc.dram_tensor("exd", [L, S * 16], F32,
                               kind="ExternalOutput")

    with tile.TileContext(nc) as tc:
        persist = tc.alloc_tile_pool(name="persist", bufs=1)

        # ---- persistent small tensors ----
        idx_t = persist.tile([128, NG], I32, name="idx_t")
        nc.sync.dma_start(idx_t[:], d_idx[:])
        wih, whh, biasR, wout = {}, {}, {}, {}
        for d in "fb":
            wih[d] = persist.tile([EMB, 4 * H], BF16, name=f"wih_{d}")
            nc.sync.dma_start(wih[d][:], d_wih[d][:])
            whh[d] = persist.tile([H, 4 * H], BF16, name=f"whh_{d}")
            nc.sync.dma_start(whh[d][:], d_whh[d][:])
            biasR[d] = persist.tile([128, 512], BF16, name=f"biasR_{d}")
            nc.sync.dma_start(biasR[d][:], d_bias[d][:])
            wout[d] = persist.tile([H, L], BF16, name=f"wout_{d}")
            nc.sync.dma_start(wout[d][:], d_wout[d][:])
        idf = persist.tile([128, 128], F32, name="idf")
        nc.sync.dma_start(idf[:], d_idf[:])
        idb = persist.tile([128, 128], BF16, name="idb")
        nc.sync.dma_start(idb[:], d_idb[:])
        teC = persist.tile([L, L], F32, name="teC_t")
        nc.sync.dma_start(teC[:], d_teC[:])
        teCT = persist.tile([L, L], F32, name="teCT_t")
        nc.sync.dma_start(teCT[:], d_teCT[:])
        estart = persist.tile([L, 1], F32, name="estart_t")
        nc.sync.dma_start(estart[:], d_estart[:])
        eend = persist.tile([L, 1], F32, name="eend_t")
        nc.sync.dma_start(eend[:], d_eend[:])
        bout = persist.tile([L, 1], F32, name="bout_t")
        nc.sync.dma_start(bout[:], d_bout[:])
        ones9 = persist.tile([L, 1], F32, name="ones9_t")
        nc.sync.dma_start(ones9[:], d_ones9[:])
        ones16 = persist.tile([16, 1], F32, name="ones16_t")
        nc.sync.dma_start(ones16[:], d_ones16[:])
        mbufA = persist.tile([16, NEVA], F32, name="mbufA")
        mbufB = persist.tile([16, NEVB], F32, name="mbufB")
        indEB = persist.tile([16, NEVB], F32, name="indEB_t")
        nc.sync.dma_start(indEB[:], d_indEB[:])
        emacc = persist.tile([L, NCH], F32, name="emacc")
        out_sb = persist.tile([1, 2], F32, name="out_sb")

        # ---- scan-side persistent buffers (allocated first: stable addrs) --
        pool_h = tc.alloc_tile_pool(name="hpool", bufs=1, side="right")
        hbuf = {d: pool_h.tile([H, NTOK], BF16, name=f"hbuf_{d}")
                for d in "fb"}
        pool_U = tc.alloc_tile_pool(name="Upool", bufs=1, side="right")
        U = {d: pool_U.tile([128, 96], F32, name=f"U_{d}") for d in "fb"}
        pool_X = tc.alloc_tile_pool(name="Xpool", bufs=3, side="right")

        # scan PSUM first so its banks never alias gather/pregemm banks
        pool_sps = tc.alloc_tile_pool(name="scanps", bufs=2, space="PSUM")

        # ======== Phase 0: gather + transpose + input pre-GEMM =============
        # full xT (both pregemm streams need chunks long after gather);
        # gx ring-buffered (8 chunks per dir, consumed in production order)
        pool_xw = tc.alloc_tile_pool(name="xtpool", bufs=1)
        xT = pool_xw.tile([128, NTOK], BF16, name="xT")
        pool_gx = tc.alloc_tile_pool(name="gxwin", bufs=8)
        pool_g = tc.alloc_tile_pool(name="gpool", bufs=4)
        pool_gp = tc.alloc_tile_pool(name="gppool", bufs=2, space="PSUM")
        pool_pgps = tc.alloc_tile_pool(name="pregemmps", bufs=2, space="PSUM")

        # interleave gather order so both scan directions can start early
        g_order = []
        lo, hi = 0, NG - 1
        while lo <= hi:
            g_order.append(lo)
            if hi != lo:
                g_order.append(hi)
            lo += 1
            hi -= 1
        gxt = {d: {} for d in "fb"}   # chunk g -> gx tile [128, 512] bf16

        def emit_gather(g):
            with nc.named_scope("p0_gather"):
                stage = pool_g.tile([128, EMB], F32, name="stage", tag="stage")
                nc.gpsimd.indirect_dma_start(
                    out=stage[:],
                    out_offset=None,
                    in_=d_emb[:],
                    in_offset=bass.IndirectOffsetOnAxis(ap=idx_t[:, g:g + 1],
                                                        axis=0),
                )
                tp = pool_gp.tile([128, 128], F32, name="tp", tag="tp")
                nc.tensor.transpose(out=tp[:], in_=stage[:], identity=idf[:])
                nc.vector.tensor_copy(out=xT[:, 128 * g:128 * (g + 1)],
                                      in_=tp[:])

        def emit_pregemm(d, g):
            with nc.named_scope("p0_pregemm"):
                pg = pool_pgps.tile([128, 512], F32, name="pg", tag="pg")
                # bias init (gate-major layout), then 4 gate blocks
                nc.tensor.matmul(out=pg[:], lhsT=idb[:], rhs=biasR[d][:],
                                 start=True, stop=False)
                for k in range(4):
                    nc.tensor.matmul(
                        out=pg[:, 128 * k:128 * (k + 1)],
                        lhsT=wih[d][:, 128 * k:128 * (k + 1)],
                        rhs=xT[:, 128 * g:128 * (g + 1)],
                        start=False, stop=(k == 3))
                gx = pool_gx.tile([128, 512], BF16, name=f"gx{d}",
                                  tag=f"gx{d}")
                gxt[d][g] = gx
                # psum (k,t,b) gate-major -> per-t layout (t,k,b)
                nc.vector.tensor_copy(
                    out=gx[:].rearrange("p (t k b) -> p k t b",
                                        t=8, k=4, b=16),
                    in_=pg[:].rearrange("p (k t b) -> p k t b",
                                        t=8, k=4, b=16))

        # warmup: gather first 12 x-chunks, pregemm first 4 per direction.
        # GWARM=12 keeps both pregemm streams >= 12 timesteps behind their
        # gathers (b-stream chunk 59-t/8 needs g_order position 2(63-c)+1).
        GWARM = min(12, NG)
        PWARM = min(4, NG // 2)
        for j in range(GWARM):
            emit_gather(g_order[j])
        for j in range(PWARM):
            emit_pregemm("f", j)
            emit_pregemm("b", NG - 1 - j)

        # ================= Phase 1: dual LSTM scan ==========================
        for d in "fb":
            nc.vector.memset(U[d][:, 64:80], 0.0)  # C = 2c = 0

        for t in range(S):
            if t % 4 == 0 and GWARM + t // 4 < NG:
                emit_gather(g_order[GWARM + t // 4])
            if t % 8 == 0:
                if PWARM + t // 8 < NG:
                    emit_pregemm("f", PWARM + t // 8)
                if NG - 1 - PWARM - t // 8 >= 0:
                    emit_pregemm("b", NG - 1 - PWARM - t // 8)
            with nc.named_scope("p1_scan"):
                for d in "fb":
                    tx = t if d == "f" else S - 1 - t  # absolute time index
                    txp = tx - 1 if d == "f" else tx + 1
                    g = tx // 8
                    gxs = gxt[d][g][:, 64 * (tx % 8):64 * (tx % 8) + 64]
                    ps = pool_sps.tile([128, 64], F32, name=f"ps_{d}",
                                       tag=f"ps{d}")
                    nc.tensor.matmul(out=ps[:], lhsT=idb[:], rhs=gxs,
                                     start=True, stop=(t == 0))
                    if t > 0:
                        hp = hbuf[d][:, 16 * txp:16 * txp + 16]
                        for k in range(4):
                            nc.tensor.matmul(
                                out=ps[:, 16 * k:16 * (k + 1)],
                                lhsT=whh[d][:, 128 * k:128 * (k + 1)],
                                rhs=hp,
                                start=False, stop=(k == 3))
                    # T' = tanh(ps) = [i' f' o' g']
                    nc.scalar.activation(U[d][:, 0:64], ps[:], ACTF.Tanh)
                    X = pool_X.tile([128, 32], F32, name="X", tag="X")
                    # X = (T'[i,f] + 1) * [g', C]
                    nc.vector.scalar_tensor_tensor(
                        out=X[:], in0=U[d][:, 0:32], scalar=1.0,
                        in1=U[d][:, 48:80], op0=ALU.add, op1=ALU.mult)
                    # C = 0.5*X_f + X_i
                    nc.vector.scalar_tensor_tensor(
                        out=U[d][:, 64:80], in0=X[:, 16:32], scalar=0.5,
                        in1=X[:, 0:16], op0=ALU.mult, op1=ALU.add)
                    # tc = tanh(0.5*C)
                    nc.scalar.activation(U[d][:, 80:96], U[d][:, 64:80],
                                         ACTF.Tanh, scale=0.5)
                    # h2 = (o' + 1) * tc
                    nc.vector.scalar_tensor_tensor(
                        out=hbuf[d][:, 16 * tx:16 * tx + 16],
                        in0=U[d][:, 32:48], scalar=1.0,
                        in1=U[d][:, 80:96], op0=ALU.add, op1=ALU.mult)

        if dump:
            nc.sync.dma_start(d_xd[:], xT[:])
        pool_pgps.release()
        pool_gp.release()
        pool_g.release()
        pool_gx.release()
        pool_xw.release()
        pool_sps.release()
        pool_X.release()
        pool_U.release()

        # ============= Phase 2: emissions + em_tag + exp ================
        pool_em = tc.alloc_tile_pool(name="empool", bufs=1)
        expem = pool_em.tile([L, NTOK], F32, name="expem")
        pool_ohm = tc.alloc_tile_pool(name="ohmpool", bufs=1)
        ohm_t = pool_ohm.tile([L, NTOK], F32, name="ohm_t")
        nc.sync.dma_start(ohm_t[:], d_ohm[:])
        pool_er = tc.alloc_tile_pool(name="emrot", bufs=2)
        pool_eps = tc.alloc_tile_pool(name="emps", bufs=2, space="PSUM")
        for c in range(NCH):
            with nc.named_scope("p2_emis"):
                sl = slice(512 * c, 512 * (c + 1))
                pe = pool_eps.tile([L, 512], F32, name="pe", tag="pe")
                nc.tensor.matmul(out=pe[:], lhsT=wout["f"][:],
                                 rhs=hbuf["f"][:, sl], start=True, stop=False)
                nc.tensor.matmul(out=pe[:], lhsT=wout["b"][:],
                                 rhs=hbuf["b"][:, sl], start=False, stop=True)
                scr = pool_er.tile([L, 512], F32, name="scr", tag="scr")
                nc.vector.tensor_tensor(out=scr[:], in0=pe[:],
                                        in1=ohm_t[:, sl], op=ALU.mult)
                nc.vector.tensor_reduce(out=emacc[:, c:c + 1], in_=scr[:],
                                        axis=AXL.X, op=ALU.add)
                nc.scalar.activation(expem[:, sl], pe[:], ACTF.Exp,
                                     bias=bout[:])

        emaccs = pool_er.tile([L, 1], F32, name="emaccs", tag="emaccs")
        nc.vector.tensor_reduce(out=emaccs[:], in_=emacc[:], axis=AXL.X,
                                op=ALU.add)
        pss = pool_eps.tile([1, 1], F32, name="pss", tag="pss")
        nc.tensor.matmul(out=pss[:], lhsT=ones9[:], rhs=emaccs[:],
                         start=True, stop=True)
        nc.vector.tensor_copy(out=out_sb[:, 0:1], in_=pss[:])

        pool_eps.release()
        pool_er.release()
        pool_ohm.release()
        if dump:
            for d in "fb":
                nc.sync.dma_start(d_hd[d][:], hbuf[d][:])
            nc.sync.dma_start(d_exd[:], expem[:])
        pool_h.release()

        # ===== Phase 3: CRF alpha/beta split scan (exp space, C-folded) =====
        MB = 16 * NB
        pool_ab = tc.alloc_tile_pool(name="abpool", bufs=1)
        Abuf = pool_ab.tile([L, MB], F32, name="Abuf")
        TBbuf = pool_ab.tile([L, MB], F32, name="TBbuf")
        Btmp = pool_ab.tile([L, MB], F32, name="Btmp")
        injbar_t = pool_ab.tile([L, MB], F32, name="injbar_t")
        einj_t = pool_ab.tile([L, MB], F32, name="einj_t")
        i255b_t = pool_ab.tile([L, 16], F32, name="i255b_t")
        ei255_t = pool_ab.tile([L, 16], F32, name="ei255_t")
        nc.sync.dma_start(injbar_t[:], d_injbar[:])
        nc.sync.dma_start(einj_t[:], d_einj[:])
        nc.sync.dma_start(i255b_t[:], d_i255b[:])
        nc.sync.dma_start(ei255_t[:], d_ei255[:])
        pool_cps = tc.alloc_tile_pool(name="crfps", bufs=2, space="PSUM")
        pool_cps1 = tc.alloc_tile_pool(name="crfps1", bufs=1, space="PSUM")
        pool_cr = tc.alloc_tile_pool(name="crfrot", bufs=3, side="right")

        with nc.named_scope("p3_pre"):
            # A = e*(1-inj);  B = e*E*inj;  TB = Te' @ B
            nc.vector.tensor_tensor(out=Abuf[:], in0=expem[:, 16 * M:],
                                    in1=injbar_t[:], op=ALU.mult)
            nc.vector.tensor_tensor(out=Btmp[:], in0=expem[:, 16 * M:],
                                    in1=einj_t[:], op=ALU.mult)
            for c in range(MB // 512):
                tb_ps = pool_cps1.tile([L, 512], F32, name="tb_ps",
                                       tag="tbps")
                nc.tensor.matmul(out=tb_ps[:], lhsT=teCT[:],
                                 rhs=Btmp[:, 512 * c:512 * (c + 1)],
                                 start=True, stop=True)
                nc.vector.tensor_copy(out=TBbuf[:, 512 * c:512 * (c + 1)],
                                      in_=tb_ps[:])

        def renorm(vtile, mtile, e, tagp):
            vt_ps = pool_cps1.tile([16, L], F32, name="vt_ps", tag="vtps")
            nc.tensor.transpose(out=vt_ps[:], in_=vtile[:],
                                identity=idf[0:L, 0:L])
            nc.vector.tensor_reduce(out=mtile[:, e:e + 1], in_=vt_ps[:],
                                    axis=AXL.X, op=ALU.max)
            rt = pool_cr.tile([16, 1], F32, name="rt", tag="rt")
            nc.vector.reciprocal(out=rt[:], in_=mtile[:, e:e + 1])
            vts = pool_cr.tile([16, L], F32, name="vts", tag="vts")
            nc.vector.tensor_scalar(out=vts[:], in0=vt_ps[:],
                                    scalar1=rt[:], scalar2=None, op0=ALU.mult)
            v2_ps = pool_cps1.tile([L, 16], F32, name="v2_ps", tag="v2ps")
            nc.tensor.transpose(out=v2_ps[:], in_=vts[:],
                                identity=idf[0:16, 0:16])
            out = pool_cr.tile([L, 16], F32, name="rn_out", tag=tagp)
            nc.vector.tensor_copy(out=out[:], in_=v2_ps[:])
            return out

        # ---- alpha chain: t = 1 .. M-1 (no masking: all lens >= M) ----
        va = pool_cr.tile([L, 16], F32, name="va", tag="va")
        nc.vector.tensor_scalar(out=va[:], in0=expem[:, 0:16],
                                scalar1=estart[:], scalar2=None, op0=ALU.mult)
        # ---- beta chain init: u = exp(end) ----
        ub0 = pool_cr.tile([L, 16], F32, name="ub0", tag="ub")
        nc.vector.memset(ub0[:], 1.0)
        ub = pool_cr.tile([L, 16], F32, name="ub", tag="ub")
        nc.vector.tensor_scalar(out=ub[:], in0=ub0[:], scalar1=eend[:],
                                scalar2=None, op0=ALU.mult)

        for step in range(1, max(M, NB + 1)):
            ta = step                 # alpha time index
            s = step - 1              # beta step index; t = S-1-s
            if ta < M:
                with nc.named_scope("p3_alpha"):
                    a_ps = pool_cps.tile([L, 16], F32, name="a_ps", tag="aps")
                    nc.tensor.matmul(out=a_ps[:], lhsT=teC[:], rhs=va[:],
                                     start=True, stop=True)
                    va2 = pool_cr.tile([L, 16], F32, name="va2", tag="va")
                    nc.vector.tensor_tensor(
                        out=va2[:], in0=a_ps[:],
                        in1=expem[:, 16 * ta:16 * ta + 16], op=ALU.mult)
                    va = va2
                    if ta % RN == RN - 1:
                        va = renorm(va, mbufA, ta // RN, "va")
            if s < NB:
                tb = S - 1 - s
                off = 16 * (tb - M)
                with nc.named_scope("p3_beta"):
                    w = pool_cr.tile([L, 16], F32, name="w", tag="wb")
                    nc.vector.tensor_tensor(out=w[:], in0=ub[:],
                                            in1=Abuf[:, off:off + 16],
                                            op=ALU.mult)
                    b_ps = pool_cps.tile([L, 16], F32, name="b_ps", tag="bps")
                    nc.tensor.matmul(out=b_ps[:], lhsT=teCT[:], rhs=w[:],
                                     start=True, stop=True)
                    ub2 = pool_cr.tile([L, 16], F32, name="ub2", tag="ub")
                    nc.vector.tensor_tensor(out=ub2[:], in0=b_ps[:],
                                            in1=TBbuf[:, off:off + 16],
                                            op=ALU.add)
                    ub = ub2
                    if s % RN == RN - 1:
                        ub = renorm(ub, mbufB, s // RN, "ub")

        pool_cps1.release()
        pool_cps.release()

        # ============= Phase 4: combine + finals ==============================
        _p4 = nc.named_scope("p4_final")
        _p4.__enter__()
        pool_f4 = tc.alloc_tile_pool(name="f4", bufs=1)
        pool_fps = tc.alloc_tile_pool(name="f4ps", bufs=2, space="PSUM")
        # boundary fix for len-1 == M-1 columns, then P = va (.) u
        uf1 = pool_f4.tile([L, 16], F32, name="uf1")
        nc.vector.tensor_tensor(out=uf1[:], in0=ub[:], in1=i255b_t[:],
                                op=ALU.mult)
        ufix = pool_f4.tile([L, 16], F32, name="ufix")
        nc.vector.tensor_tensor(out=ufix[:], in0=uf1[:], in1=ei255_t[:],
                                op=ALU.add)
        P = pool_f4.tile([L, 16], F32, name="P")
        nc.vector.tensor_tensor(out=P[:], in0=va[:], in1=ufix[:],
                                op=ALU.mult)
        w_ps = pool_fps.tile([1, 16], F32, name="w_ps", tag="wps")
        nc.tensor.matmul(out=w_ps[:], lhsT=ones9[:], rhs=P[:],
                         start=True, stop=True)
        lw = pool_f4.tile([1, 16], F32, name="lw")
        nc.scalar.activation(lw[:], w_ps[:], ACTF.Ln)
        lwT_ps = pool_fps.tile([16, 1], F32, name="lwT_ps", tag="lwT")
        nc.tensor.transpose(out=lwT_ps[:], in_=lw[:],
                            identity=idf[0:1, 0:1])
        lnA = pool_f4.tile([16, NEVA], F32, name="lnA")
        nc.scalar.activation(lnA[:], mbufA[:], ACTF.Ln)
        redA = pool_f4.tile([16, 1], F32, name="redA")
        nc.vector.tensor_reduce(out=redA[:], in_=lnA[:], axis=AXL.X,
                                op=ALU.add)
        lnB = pool_f4.tile([16, NEVB], F32, name="lnB")
        nc.scalar.activation(lnB[:], mbufB[:], ACTF.Ln)
        lnBm = pool_f4.tile([16, NEVB], F32, name="lnBm")
        nc.vector.tensor_tensor(out=lnBm[:], in0=lnB[:], in1=indEB[:],
                                op=ALU.mult)
        redB = pool_f4.tile([16, 1], F32, name="redB")
        nc.vector.tensor_reduce(out=redB[:], in_=lnBm[:], axis=AXL.X,
                                op=ALU.add)
        dst = pool_f4.tile([16, 1], F32, name="dst")
        nc.vector.tensor_tensor(out=dst[:], in0=lwT_ps[:], in1=redA[:],
                                op=ALU.add)
        dst2 = pool_f4.tile([16, 1], F32, name="dst2")
        nc.vector.tensor_tensor(out=dst2[:], in0=dst[:], in1=redB[:],
                                op=ALU.add)
        dtot_ps = pool_fps.tile([1, 1], F32, name="dtot_ps", tag="dtot")
        nc.tensor.matmul(out=dtot_ps[:], lhsT=ones16[:], rhs=dst2[:],
                         start=True, stop=True)
        nc.vector.tensor_copy(out=out_sb[:, 1:2], in_=dtot_ps[:])
        pool_fps.release()
        pool_f4.release()
        _p4.__exit__(None, None, None)
        pool_cr.release()
        pool_ab.release()
        pool_em.release()

        nc.sync.dma_start(d_out[:], out_sb[:])
        persist.release()

    nc.compile()
    return nc


# ---------------------------------------------------------------------------
# Host side
# ---------------------------------------------------------------------------

def _prep_core_inputs(core, seqs, labels, start_t, end_t, trans, b_out,
                      S, BL, RN, lnC, shared):
    NTOK = S * BL
    NG = NTOK // 128
    M = S // 2
    NB = S - M
    NEVB = NB // RN
    b0 = core * BL
    sq = seqs[b0:b0 + BL]          # [BL, S]
    lb = labels[b0:b0 + BL]
    lens = (sq != PAD).sum(axis=1).astype(np.int64)
    maskf = (sq != PAD).astype(np.float32)

    # token gather indices in (t, b) order
    toks = sq.T.reshape(-1).astype(np.int32)       # [S*BL], t-major
    idx = np.ascontiguousarray(toks.reshape(NG, 128).T)

    ohm = np.zeros((L, NTOK), np.float32)
    cols = np.arange(NTOK)
    t_of = cols // BL
    b_of = cols % BL
    ohm[lb[b_of, t_of], cols] = maskf[b_of, t_of]

    # beta-chain injection tensors: inj_t[b] = (len_b-1 == t), t in [M, S-1]
    E = np.exp(end_t.astype(np.float32))           # [L]
    ts = np.arange(M, S)
    inj = (lens[None, :] - 1 == ts[:, None]).astype(np.float32)  # [NB, BL]
    injbar = np.ascontiguousarray(
        np.broadcast_to((1.0 - inj).reshape(1, -1), (L, 16 * NB)).astype(
            np.float32))
    einj = np.ascontiguousarray(
        (E[:, None, None] * inj[None]).reshape(L, 16 * NB).astype(np.float32))
    i255 = (lens - 1 == M - 1).astype(np.float32)  # [BL]
    i255b = np.ascontiguousarray(
        np.broadcast_to((1.0 - i255)[None, :], (L, 16)).astype(np.float32))
    ei255 = np.ascontiguousarray(
        (E[:, None] * i255[None, :]).astype(np.float32))
    # beta renorm event at s=e*RN+RN-1 -> t_e = S-1-s; survives iff
    # t_e <= len_b-1
    s_e = np.arange(NEVB) * RN + RN - 1
    t_e = S - 1 - s_e
    indEB = np.ascontiguousarray(
        (t_e[None, :] <= (lens - 1)[:, None]).astype(np.float32))

    inmap = dict(shared)
    inmap["idx"] = idx
    inmap["ohm"] = ohm
    inmap["injbar"] = injbar
    inmap["einj"] = einj
    inmap["i255b"] = i255b
    inmap["ei255"] = ei255
    inmap["indEB"] = indEB

    ar = np.arange(BL)
    hostnum = (start_t[lb[:, 0]]
               + (trans[lb[:, :-1], lb[:, 1:]] * maskf[:, 1:]).sum(axis=1)
               + end_t[lb[ar, lens - 1]]
               + (maskf * b_out[lb]).sum(axis=1))
    # C-fold correction: device denom includes (len-1)*lnC extra
    hostnum_total = float(hostnum.sum()) + lnC * float((lens - 1).sum())
    return inmap, hostnum_total


def _shared_inputs(emb, w_ih, w_hh, b_ih, b_hh, w_out, b_out, start_t,
                   end_t, trans):
    # pytorch gate rows [i, f, g, o] -> device gate blocks [i, f, o, g]
    perm = [0, 1, 3, 2]
    # tanh-only gates: i,f,o pre-scaled by 1/2 (sigmoid via tanh); h stored
    # as h2=2h so all W_hh contributions halved again.
    sc_ih = [0.5, 0.5, 0.5, 1.0]
    sc_hh = [0.25, 0.25, 0.25, 0.5]

    def wprep(w, scales):  # [4H, K] -> [K, 4H] col blocks in perm order
        blocks = [w[128 * p:128 * (p + 1)].T * s
                  for p, s in zip(perm, scales)]
        return np.ascontiguousarray(
            np.concatenate(blocks, axis=1)).astype(ml_dtypes.bfloat16)

    def bprep(bi, bh):
        bsum = (bi + bh).astype(np.float32)
        blocks = [np.repeat(bsum[128 * p:128 * (p + 1)][:, None] * s,
                            128, axis=1)
                  for p, s in zip(perm, sc_ih)]
        return np.ascontiguousarray(
            np.concatenate(blocks, axis=1)).astype(ml_dtypes.bfloat16)

    Te = np.exp(trans.astype(np.float64))
    C = float(1.0 / (L * Te.mean() * np.exp(b_out.astype(np.float64)).mean()))
    lnC = float(np.log(C))
    te9C = (C * Te).astype(np.float32)

    shared = {
        "emb": np.ascontiguousarray(emb, dtype=np.float32),
        "ident_f32": np.eye(128, dtype=np.float32),
        "ident_bf16": np.eye(128).astype(ml_dtypes.bfloat16),
        "te9C": np.ascontiguousarray(te9C),
        "te9CT": np.ascontiguousarray(te9C.T),
        "expstart": np.exp(start_t.astype(np.float32))[:, None].copy(),
        "expend": np.exp(end_t.astype(np.float32))[:, None].copy(),
        "bout9": b_out.astype(np.float32)[:, None].copy(),
        "ones9": np.ones((L, 1), np.float32),
        "ones16": np.ones((16, 1), np.float32),
    }
    for d in "fb":
        shared[f"wihT_{d}"] = wprep(w_ih[d], sc_ih)
        shared[f"whhT_{d}"] = wprep(w_hh[d], sc_hh)
        shared[f"biasR_{d}"] = bprep(b_ih[d], b_hh[d])
    shared["woutT_f"] = np.ascontiguousarray(
        0.5 * w_out[:, :H].T).astype(ml_dtypes.bfloat16)
    shared["woutT_b"] = np.ascontiguousarray(
        0.5 * w_out[:, H:].T).astype(ml_dtypes.bfloat16)
    return shared, lnC


_CACHE = {}


def run(inputs, S=S_FULL, BL=16, RN=None, n_cores=N_CORES_FULL, dump=False,
        **spmd_kwargs):
    seqs = np.asarray(inputs["sequences"])
    labels = np.asarray(inputs["labels"])
    emb = np.asarray(inputs["emb"], np.float32)
    w_ih = {"f": np.asarray(inputs["w_ih_f"], np.float32),
            "b": np.asarray(inputs["w_ih_b"], np.float32)}
    w_hh = {"f": np.asarray(inputs["w_hh_f"], np.float32),
            "b": np.asarray(inputs["w_hh_b"], np.float32)}
    b_ih = {"f": np.asarray(inputs["b_ih_f"], np.float32),
            "b": np.asarray(inputs["b_ih_b"], np.float32)}
    b_hh = {"f": np.asarray(inputs["b_hh_f"], np.float32),
            "b": np.asarray(inputs["b_hh_b"], np.float32)}
    w_out = np.asarray(inputs["w_out"], np.float32)
    b_out = np.asarray(inputs["b_out"], np.float32)
    start_t = np.asarray(inputs["start_t"], np.float32)
    end_t = np.asarray(inputs["end_t"], np.float32)
    trans = np.asarray(inputs["trans"], np.float32)

    if RN is None:
        RN = 64 if S >= 512 else 16

    key = (S, BL, RN, dump)
    if key not in _CACHE:
        _CACHE[key] = build_nc(S=S, BL=BL, RN=RN, dump=dump)
    nc = _CACHE[key]

    shared, lnC = _shared_inputs(emb, w_ih, w_hh, b_ih, b_hh, w_out, b_out,
                                 start_t, end_t, trans)
    in_maps = []
    hostnum_total = 0.0
    for c in range(n_cores):
        im, hn = _prep_core_inputs(c, seqs, labels, start_t, end_t, trans,
                                   b_out, S, BL, RN, lnC, shared)
        in_maps.append(im)
        hostnum_total += hn

    res = bass_utils.run_bass_kernel_spmd(nc, in_maps,
                                          core_ids=list(range(n_cores)),
                                          **spmd_kwargs)
    emtag_total = 0.0
    denom_total = 0.0
    for r in res.results:
        emtag_total += float(r["out2"][0, 0])
        denom_total += float(r["out2"][0, 1])
    loss = denom_total - (hostnum_total + emtag_total)
    return np.array(loss, dtype=np.float32), res


def kernel(**inputs):
    loss, _ = run(inputs)
    return loss
